# revision 2
# baseline (speedup 1.0000x reference)
"""Trainium2 Bass kernel for EnhancedAttention (B=2, T=2048, D=1024, H=16, DH=64).

Sharding: 8 cores = 2 batches x 4 head-groups (4 heads each). No collectives;
each core computes a partial out-projection and the host sums the 4 partials
per batch.

v2 redesign vs baseline:
  - S matmuls for the two heads of a pair are emitted back-to-back with
    base partitions 0/64 so they run CONCURRENTLY in different PE row groups.
  - vaug carries 64 ones-columns so the O' matmul replicates the softmax
    denominator across partitions 64:128; normalization is then just a
    [64,512] DVE reciprocal + [64,512] DVE multiply straight out of PSUM
    (no 1-partition reciprocal, no partition_broadcast, no extract DMA).
  - exp runs only on the causally-live region; the causal mask is applied to
    just the 128-col diagonal block.
  - All dma_starts dispatch from sync/gpsimd so the Scalar engine does
    nothing but exp (it is ~70us of line-rate work, near critical).
  - proj / yproj work units are interleaved into the attention kt-loops so
    the PE never idles while ACT streams exps (keeps HAM at K=8/8).
"""
import os
import sys

for _p in ("/opt/trn_rl_repo", "/root/.axon_site/_ro/trn_rl_repo"):
    if os.path.isdir(_p) and _p not in sys.path:
        sys.path.append(_p)

import ml_dtypes
import numpy as np

import concourse.bass as bass  # noqa: F401
import concourse.tile as tile
from concourse import bacc, mybir
from concourse.bass_utils import run_bass_kernel_spmd

B, T, D = 2, 2048, 1024
H, DH = 16, 64
HPC = 4  # heads per core
NCORES = 8
ROPE_THETA = 10000.0

F32 = mybir.dt.float32
BF16 = mybir.dt.bfloat16

TCH = 512  # t-chunk (q-chunk) size
TC = T // TCH  # 4
DC = D // 128  # 8 contraction chunks
NKT = T // 128  # 16 k-tiles


def _rope_tables():
    inv = 1.0 / (ROPE_THETA ** (np.arange(0, DH, 2, dtype=np.float64) / DH))
    f = np.arange(T, dtype=np.float64)[:, None] * inv[None, :]  # [T, 32]
    cos = np.cos(f).T.astype(ml_dtypes.bfloat16)  # [32, T]
    sin = np.sin(f).T.astype(ml_dtypes.bfloat16)
    cs1 = np.ascontiguousarray(np.tile(cos, (4, 1)))  # [128, T]
    # sign pattern chosen so that after the 32-strip swap the sin terms land
    # as [-sin*x2, sin*x1] against the cos terms
    cs2 = np.ascontiguousarray(np.concatenate([sin, -sin, sin, -sin], axis=0))
    return cs1, cs2


def _build():
    nc = bacc.Bacc("TRN2", target_bir_lowering=False, debug=False, num_devices=NCORES)
    xT_d = nc.dram_tensor("xT", [D, T], BF16, kind="ExternalInput")
    wq_d = nc.dram_tensor("wq", [D, HPC * DH], BF16, kind="ExternalInput")
    wk_d = nc.dram_tensor("wk", [D, HPC * DH], BF16, kind="ExternalInput")
    wv_d = nc.dram_tensor("wv", [D, HPC * DH], BF16, kind="ExternalInput")
    wo_d = nc.dram_tensor("wo", [HPC * DH, D], BF16, kind="ExternalInput")
    y_d = nc.dram_tensor("y", [T, D], F32, kind="ExternalOutput")

    cs1_np, cs2_np = _rope_tables()
    cs1_d = nc.inline_tensor(cs1_np, "cs1")
    cs2_d = nc.inline_tensor(cs2_np, "cs2")

    EXP = mybir.ActivationFunctionType.Exp

    import contextlib
    with tile.TileContext(nc) as tc:
        with (
            contextlib.ExitStack() as _ctx,
            tc.tile_pool(name="sb", bufs=1) as sb,
            tc.tile_pool(name="xtp", bufs=2) as xtp,
            tc.tile_pool(name="ropep", bufs=2) as ropep,
            tc.tile_pool(name="ptp", bufs=4) as ptp,
            tc.tile_pool(name="recp", bufs=3) as recp,
            tc.tile_pool(name="ysbp", bufs=3) as ysbp,
        ):
            wq = sb.tile([128, DC, HPC * DH], BF16)
            wk = sb.tile([128, DC, HPC * DH], BF16)
            wv = sb.tile([128, DC, HPC * DH], BF16)
            wo = sb.tile([128, 2, D], BF16)
            cs1 = sb.tile([128, T], BF16)
            cs2 = sb.tile([128, T], BF16)
            qt = [sb.tile([128, T], BF16, tag=f"qt{p}", name=f"qt{p}") for p in range(2)]
            ktt = [sb.tile([128, T], BF16, tag=f"kt{p}", name=f"kt{p}") for p in range(2)]
            # per (ktile, head): [ones (64) | V (64 dh)] -> O' matmul yields
            # softmax denominator on partitions 0:64 (base-0 for the custom
            # fast-reciprocal op), attention numerator on 64:128
            vaug = sb.tile([128, NKT, HPC, 2 * DH], BF16)
            ot = [sb.tile([128, T], BF16, tag=f"ot{p}", name=f"ot{p}") for p in range(2)]

            # input DMAs: wq + x0 + cs first on sync (needed first), rest gpsimd
            nc.sync.dma_start(wq[:], wq_d.ap().rearrange("(c p) n -> p c n", p=128))
            nc.gpsimd.dma_start(wk[:], wk_d.ap().rearrange("(c p) n -> p c n", p=128))
            nc.gpsimd.dma_start(wv[:], wv_d.ap().rearrange("(c p) n -> p c n", p=128))
            nc.gpsimd.dma_start(wo[:], wo_d.ap().rearrange("(c p) n -> p c n", p=128))
            nc.gpsimd.memset(vaug[:, :, :, 0:DH], 1.0)

            xT_r = xT_d.ap().rearrange("(c p) t -> p c t", p=128)
            xts = {}

            def x_dma(tci):
                tsl = slice(tci * TCH, (tci + 1) * TCH)
                xt = xtp.tile([128, DC, TCH], BF16, tag="xt", name=f"xt{tci}")
                xts[tci] = xt
                nc.sync.dma_start(xt[:, 0:4, :], xT_r[:, 0:4, tsl])
                nc.sync.dma_start(xt[:, 4:8, :], xT_r[:, 4:8, tsl])

            x_dma(0)
            nc.sync.dma_start(cs1[:], cs1_d.ap())
            nc.sync.dma_start(cs2[:], cs2_d.ap())
            x_dma(1)

            pjps = _ctx.enter_context(tc.tile_pool(name="pjps", bufs=2, space="PSUM"))
            sps = _ctx.enter_context(tc.tile_pool(name="sps", bufs=2, space="PSUM"))
            ops = _ctx.enter_context(tc.tile_pool(name="ops", bufs=2, space="PSUM"))

            # PE warm-up: fills the input-DMA wait so HAM reaches K=8/8
            warm = sb.tile([128, TCH], BF16, name="warm")
            nc.vector.memset(warm, 0.0)
            wps = pjps.tile([128, TCH], F32, tag="pj", name="wps")
            for wi in range(10):
                nc.tensor.matmul(
                    wps[:], warm[:, 0:128], warm[:],
                    start=(wi == 0), stop=(wi == 9),
                )

            rope_bufs = {}

            def qk_unit(tci, w_sb, dest, p, u):
                # u = 0..3 over (q_p0, q_p1, k_p0, k_p1); last one does the
                # batched strip-swap + adds for the whole chunk
                tsl = slice(tci * TCH, (tci + 1) * TCH)
                xt = xts[tci]
                ps = pjps.tile([128, TCH], F32, tag="pj", name=f"pj{tci}_{u}")
                for dc in range(DC):
                    nc.tensor.matmul(
                        ps[:],
                        w_sb[:, dc, p * 128 : (p + 1) * 128],
                        xt[:, dc, :],
                        start=(dc == 0),
                        stop=(dc == DC - 1),
                    )
                if u == 0:
                    t1 = ropep.tile([128, 4, TCH], BF16, tag="t1", name=f"t1_{tci}")
                    t2 = ropep.tile([128, 4, TCH], BF16, tag="t2", name=f"t2_{tci}")
                    swt = ropep.tile([128, 4, TCH], BF16, tag="swt", name=f"sw_{tci}")
                    rope_bufs[tci] = (t1, t2, swt)
                else:
                    t1, t2, swt = rope_bufs[tci]
                nc.vector.tensor_mul(t1[:, u, :], ps[:], cs1[:, tsl])
                nc.vector.tensor_mul(t2[:, u, :], ps[:], cs2[:, tsl])
                if u == 3:
                    # 32-strip swap (0<->1, 2<->3) across all 4 (q/k, p) slots
                    for s in range(4):
                        nc.sync.dma_start(
                            swt[s * 32 : (s + 1) * 32, :, :],
                            t2[(s ^ 1) * 32 : ((s ^ 1) + 1) * 32, :, :],
                        )
                    for uu, (d, pp) in enumerate(
                        ((qt, 0), (qt, 1), (ktt, 0), (ktt, 1))
                    ):
                        nc.vector.tensor_add(
                            d[pp][:, tsl], t1[:, uu, :], swt[:, uu, :]
                        )

            def v_unit(tci, tt):
                gt = tci * 4 + tt
                xt = xts[tci]
                ps = pjps.tile([128, HPC * DH], F32, tag="pj", name=f"pjv{gt}")
                for dc in range(DC):
                    nc.tensor.matmul(
                        ps[:],
                        xt[:, dc, tt * 128 : (tt + 1) * 128],
                        wv[:, dc, :],
                        start=(dc == 0),
                        stop=(dc == DC - 1),
                    )
                nc.vector.tensor_copy(
                    vaug[:, gt, :, DH:],
                    ps[:].rearrange("p (h d) -> p h d", h=HPC),
                )

            def proj_units(tci):
                return [
                    (lambda p=p, w=w, u=u: qk_unit(tci, w, None, p, u))
                    for u, (w, p) in enumerate(
                        ((wq, 0), (wq, 1), (wk, 0), (wk, 1))
                    )
                ] + [(lambda tt=tt: v_unit(tci, tt)) for tt in range(4)]

            def yproj_unit(qc, tt, ni):
                gtt = qc * 4 + tt
                yps = pjps.tile([128, TCH], F32, tag="pj", name=f"y{gtt}_{ni}")
                for p2 in range(2):
                    nc.tensor.matmul(
                        yps[:],
                        ot[p2][:, gtt * 128 : (gtt + 1) * 128],
                        wo[:, p2, ni * TCH : (ni + 1) * TCH],
                        start=(p2 == 0),
                        stop=(p2 == 1),
                    )
                ysb = ysbp.tile([128, TCH], F32, tag="ysb", name=f"ys{gtt}_{ni}")
                nc.vector.tensor_copy(ysb[:], yps[:])
                nc.sync.dma_start(
                    y_d.ap()[gtt * 128 : (gtt + 1) * 128, ni * TCH : (ni + 1) * TCH],
                    ysb[:],
                )

            def yproj_units(qc):
                return [
                    (lambda tt=tt, ni=ni: yproj_unit(qc, tt, ni))
                    for tt in range(4)
                    for ni in range(2)
                ]

            OD = 2  # O' trails S by this many k-tiles

            def attn_chunk(qc, units):
                nkt = 4 * qc + 4
                total_iters = 2 * nkt
                emitted = [0]
                it = [0]

                def fill():
                    it[0] += 1
                    want = len(units) * it[0] // total_iters
                    while emitted[0] < want:
                        units[emitted[0]]()
                        emitted[0] += 1

                for hp in range(2):
                    h0, h1 = 2 * hp, 2 * hp + 1
                    opsA = ops.tile([128, TCH], F32, tag="o", name=f"oA{qc}_{hp}")
                    opsB = ops.tile([128, TCH], F32, tag="o", name=f"oB{qc}_{hp}")
                    live = {}

                    def s_step(kt):
                        off = max(0, 128 * kt - 512 * qc)
                        sp = sps.tile([128, 2, TCH], F32, tag="s", name=f"s{qc}_{hp}_{kt}")
                        pt = ptp.tile([128, 2, TCH], BF16, tag="pt", name=f"pt{qc}_{hp}_{kt}")
                        live[kt] = (pt, off)
                        for j, h in ((0, h0), (1, h1)):
                            pb = 64 * j
                            nc.tensor.matmul(
                                sp[:, j, off:],
                                ktt[hp][pb : pb + 64, kt * 128 : (kt + 1) * 128],
                                qt[hp][pb : pb + 64, qc * 512 + off : (qc + 1) * 512],
                                start=True,
                                stop=True,
                            )
                        nc.scalar.activation(
                            pt[:, :, off:], sp[:, :, off:], EXP, bias=0.0, scale=0.125
                        )
                        if kt >= 4 * qc:
                            nc.gpsimd.affine_select(
                                out=pt[:, :, off : off + 128],
                                in_=pt[:, :, off : off + 128],
                                compare_op=mybir.AluOpType.is_ge,
                                fill=0.0,
                                base=0,
                                pattern=[[0, 2], [1, 128]],
                                channel_multiplier=-1,
                            )

                    def o_step(kt):
                        pt, off = live.pop(kt)
                        for j, h in ((0, h0), (1, h1)):
                            dst = opsA if j == 0 else opsB
                            nc.tensor.matmul(
                                dst[:, off:],
                                vaug[:, kt, h, :],
                                pt[:, j, off:],
                                start=(kt == 0),
                                stop=(kt == nkt - 1),
                            )

                    for kt in range(nkt):
                        s_step(kt)
                        if kt >= OD:
                            o_step(kt - OD)
                        fill()
                    for kt in range(nkt - OD, nkt):
                        o_step(kt)

                    qsl = slice(qc * 512, (qc + 1) * 512)
                    recA = recp.tile([64, TCH], F32, tag="rec", name=f"rA{qc}_{hp}")
                    nc.vector.reciprocal_approx_fast(recA[:], opsA[0:64, :])
                    nc.vector.tensor_mul(ot[hp][0:64, qsl], opsA[64:128, :], recA[:])
                    recB = recp.tile([64, TCH], F32, tag="rec", name=f"rB{qc}_{hp}")
                    nc.vector.reciprocal_approx_fast(recB[:], opsB[0:64, :])
                    nc.vector.tensor_mul(ot[hp][64:128, qsl], opsB[64:128, :], recB[:])

                while emitted[0] < len(units):
                    units[emitted[0]]()
                    emitted[0] += 1

            # dense proj for chunk 0, then attention chunks with interleave
            for u in proj_units(0):
                u()
            attn_chunk(0, [lambda: x_dma(2)] + proj_units(1))
            attn_chunk(1, [lambda: x_dma(3)] + proj_units(2) + yproj_units(0))
            attn_chunk(2, proj_units(3) + yproj_units(1))
            attn_chunk(3, yproj_units(2))
            for u in yproj_units(3):
                u()
    nc.compile()
    return nc


_NC_CACHE = []


def _get_nc():
    if not _NC_CACHE:
        _NC_CACHE.append(_build())
    return _NC_CACHE[0]


_LAST_RESULTS = []  # stashed BassKernelResults for test harness introspection


def kernel(x, Wqkv, Wout, _trace=False, **_trace_kwargs):
    x = np.asarray(x, dtype=np.float32)
    Wqkv = np.asarray(Wqkv, dtype=np.float32)
    Wout = np.asarray(Wout, dtype=np.float32)

    nc = _get_nc()
    in_maps = []
    for c in range(NCORES):
        b, g = divmod(c, HPC)
        cols = slice(g * HPC * DH, (g + 1) * HPC * DH)
        rows = slice(g * HPC * DH, (g + 1) * HPC * DH)
        bf = ml_dtypes.bfloat16
        in_maps.append(
            {
                "xT": np.ascontiguousarray(x[b].T.astype(bf)),
                "wq": np.ascontiguousarray(Wqkv[:, 0:D][:, cols].astype(bf)),
                "wk": np.ascontiguousarray(Wqkv[:, D : 2 * D][:, cols].astype(bf)),
                "wv": np.ascontiguousarray(Wqkv[:, 2 * D : 3 * D][:, cols].astype(bf)),
                "wo": np.ascontiguousarray(Wout[rows, :].astype(bf)),
            }
        )

    res = run_bass_kernel_spmd(
        nc, in_maps, core_ids=list(range(NCORES)), trace=_trace, **_trace_kwargs
    )
    _LAST_RESULTS.clear()
    _LAST_RESULTS.append(res)

    out = np.zeros((B, T, D), dtype=np.float32)
    for c in range(NCORES):
        b = c // HPC
        out[b] += res.results[c]["y"]
    return out


# revision 3
# speedup vs baseline: 1.0166x; 1.0166x over previous
"""Trainium2 Bass kernel for EnhancedAttention (B=2, T=2048, D=1024, H=16, DH=64).

Sharding: 8 cores = 2 batches x 4 head-groups (4 heads each). No collectives;
each core computes a partial out-projection and the host sums the 4 partials
per batch.

v2 redesign vs baseline:
  - S matmuls for the two heads of a pair are emitted back-to-back with
    base partitions 0/64 so they run CONCURRENTLY in different PE row groups.
  - vaug carries 64 ones-columns so the O' matmul replicates the softmax
    denominator across partitions 64:128; normalization is then just a
    [64,512] DVE reciprocal + [64,512] DVE multiply straight out of PSUM
    (no 1-partition reciprocal, no partition_broadcast, no extract DMA).
  - exp runs only on the causally-live region; the causal mask is applied to
    just the 128-col diagonal block.
  - All dma_starts dispatch from sync/gpsimd so the Scalar engine does
    nothing but exp (it is ~70us of line-rate work, near critical).
  - proj / yproj work units are interleaved into the attention kt-loops so
    the PE never idles while ACT streams exps (keeps HAM at K=8/8).
"""
import os
import sys

for _p in ("/opt/trn_rl_repo", "/root/.axon_site/_ro/trn_rl_repo"):
    if os.path.isdir(_p) and _p not in sys.path:
        sys.path.append(_p)

import ml_dtypes
import numpy as np

import concourse.bass as bass  # noqa: F401
import concourse.tile as tile
from concourse import bacc, mybir
from concourse.bass_utils import run_bass_kernel_spmd

B, T, D = 2, 2048, 1024
H, DH = 16, 64
HPC = 4  # heads per core
NCORES = 8
ROPE_THETA = 10000.0

F32 = mybir.dt.float32
BF16 = mybir.dt.bfloat16
FP16 = mybir.dt.float16

TCH = 512  # t-chunk (q-chunk) size
TC = T // TCH  # 4
DC = D // 128  # 8 contraction chunks
NKT = T // 128  # 16 k-tiles


def _rope_tables():
    inv = 1.0 / (ROPE_THETA ** (np.arange(0, DH, 2, dtype=np.float64) / DH))
    f = np.arange(T, dtype=np.float64)[:, None] * inv[None, :]  # [T, 32]
    cos = np.cos(f).T.astype(np.float16)  # [32, T]
    sin = np.sin(f).T.astype(np.float16)
    cs1 = np.ascontiguousarray(np.tile(cos, (4, 1)))  # [128, T]
    # sign pattern chosen so that after the 32-strip swap the sin terms land
    # as [-sin*x2, sin*x1] against the cos terms
    cs2 = np.ascontiguousarray(np.concatenate([sin, -sin, sin, -sin], axis=0))
    return cs1, cs2


def _build():
    nc = bacc.Bacc("TRN2", target_bir_lowering=False, debug=False, num_devices=NCORES)
    # all inputs pre-arranged on host to [128 partitions, ...] contiguous
    x_d = nc.dram_tensor("xp", [128, DC * T], BF16, kind="ExternalInput")
    wq_d = nc.dram_tensor("wq", [128, DC * HPC * DH], BF16, kind="ExternalInput")
    wk_d = nc.dram_tensor("wk", [128, DC * HPC * DH], BF16, kind="ExternalInput")
    wv_d = nc.dram_tensor("wv", [128, DC * HPC * DH], BF16, kind="ExternalInput")
    wo_d = nc.dram_tensor("wo", [128, 2 * D], BF16, kind="ExternalInput")
    y_d = nc.dram_tensor("y", [T, D], F32, kind="ExternalOutput")

    cs1_np, cs2_np = _rope_tables()
    cs1_d = nc.inline_tensor(cs1_np, "cs1")
    cs2_d = nc.inline_tensor(cs2_np, "cs2")

    EXP = mybir.ActivationFunctionType.Exp

    import contextlib
    with tile.TileContext(nc) as tc:
        with (
            contextlib.ExitStack() as _ctx,
            tc.tile_pool(name="sb", bufs=1) as sb,
            tc.tile_pool(name="ropep", bufs=2) as ropep,
            tc.tile_pool(name="ptp", bufs=4) as ptp,
            tc.tile_pool(name="recp", bufs=3) as recp,
            tc.tile_pool(name="ysbp", bufs=3) as ysbp,
        ):
            wq = sb.tile([128, DC, HPC * DH], BF16)
            wk = sb.tile([128, DC, HPC * DH], BF16)
            wv = sb.tile([128, DC, HPC * DH], BF16)
            wo = sb.tile([128, 2, D], BF16)
            cs1 = sb.tile([128, T], FP16)
            cs2 = sb.tile([128, T], FP16)
            qt = [sb.tile([128, T], BF16, tag=f"qt{p}", name=f"qt{p}") for p in range(2)]
            ktt = [sb.tile([128, T], BF16, tag=f"kt{p}", name=f"kt{p}") for p in range(2)]
            # per (ktile, head): [ones (64) | V (64 dh)] -> O' matmul yields
            # softmax denominator on partitions 0:64 (base-0 for the custom
            # fast-reciprocal op), attention numerator on 64:128
            vaug = sb.tile([128, NKT, HPC, 2 * DH], BF16)
            ot = [sb.tile([128, T], BF16, tag=f"ot{p}", name=f"ot{p}") for p in range(2)]

            # whole x resident in SBUF; all DRAM reads are contiguous.
            # spread the 7MB of input loads across 4 HWDGE queues; gpsimd
            # (slow SWDGE) only gets wo which isn't needed until ~60us.
            xt = sb.tile([128, DC, T], BF16, name="xt")
            x_r = x_d.ap().rearrange("p (c t) -> p c t", c=DC)
            nc.sync.dma_start(wq[:], wq_d.ap().rearrange("p (c n) -> p c n", c=DC))
            nc.scalar.dma_start(wk[:], wk_d.ap().rearrange("p (c n) -> p c n", c=DC))
            nc.sync.dma_start(xt[:, 0:2, :], x_r[:, 0:2, :])
            nc.scalar.dma_start(xt[:, 4:6, :], x_r[:, 4:6, :])
            nc.sync.dma_start(cs1[:], cs1_d.ap())
            nc.scalar.dma_start(cs2[:], cs2_d.ap())
            nc.sync.dma_start(xt[:, 2:4, :], x_r[:, 2:4, :])
            nc.scalar.dma_start(xt[:, 6:8, :], x_r[:, 6:8, :])
            nc.sync.dma_start(wv[:], wv_d.ap().rearrange("p (c n) -> p c n", c=DC))
            nc.gpsimd.dma_start(wo[:], wo_d.ap().rearrange("p (c n) -> p c n", c=2))
            nc.gpsimd.memset(vaug[:, :, :, 0:DH], 1.0)

            pjps = _ctx.enter_context(tc.tile_pool(name="pjps", bufs=2, space="PSUM"))
            sps = _ctx.enter_context(tc.tile_pool(name="sps", bufs=2, space="PSUM"))
            ops = _ctx.enter_context(tc.tile_pool(name="ops", bufs=2, space="PSUM"))

            # PE warm-up: fills the input-DMA wait so HAM reaches K=8/8
            warm = sb.tile([128, TCH], BF16, name="warm")
            nc.vector.memset(warm, 0.0)
            wps = pjps.tile([128, TCH], F32, tag="pj", name="wps")
            for wi in range(6):
                nc.tensor.matmul(
                    wps[:, 0:256], warm[:, 0:128], warm[:, 0:256],
                    start=(wi == 0), stop=(wi == 5),
                )

            rope_bufs = {}

            def qk_unit(tci, w_sb, p, wi):
                # wi=0: Q projection (allocates the pair's rope tiles);
                # wi=1: K projection + strip-swap + adds -> qt[p]/ktt[p] ready
                tsl = slice(tci * TCH, (tci + 1) * TCH)
                ps = pjps.tile([128, TCH], F32, tag="pj", name=f"pj{tci}_{p}_{wi}")
                for dc in range(DC):
                    nc.tensor.matmul(
                        ps[:],
                        w_sb[:, dc, p * 128 : (p + 1) * 128],
                        xt[:, dc, tsl],
                        start=(dc == 0),
                        stop=(dc == DC - 1),
                    )
                if wi == 0:
                    t1 = ropep.tile([128, 2, TCH], FP16, tag="t1", name=f"t1_{tci}_{p}")
                    t2 = ropep.tile([128, 2, TCH], FP16, tag="t2", name=f"t2_{tci}_{p}")
                    swt = ropep.tile([128, 2, TCH], FP16, tag="swt", name=f"sw_{tci}_{p}")
                    rope_bufs[(tci, p)] = (t1, t2, swt)
                else:
                    t1, t2, swt = rope_bufs[(tci, p)]
                # fast PSUM evac (frees the pjps slot), then 16-bit DVE math
                pe = ropep.tile([128, TCH], FP16, tag="pe", name=f"pe{tci}_{p}_{wi}")
                nc.vector.tensor_copy(pe[:], ps[:])
                nc.vector.tensor_mul(t1[:, wi, :], pe[:], cs1[:, tsl])
                nc.vector.tensor_mul(t2[:, wi, :], pe[:], cs2[:, tsl])
                if wi == 1:
                    # 32-strip swap (0<->1, 2<->3) across the (q, k) pair
                    for s in range(4):
                        nc.sync.dma_start(
                            swt[s * 32 : (s + 1) * 32, :, :],
                            t2[(s ^ 1) * 32 : ((s ^ 1) + 1) * 32, :, :],
                        )
                    nc.vector.tensor_add(qt[p][:, tsl], t1[:, 0, :], swt[:, 0, :])
                    nc.vector.tensor_add(ktt[p][:, tsl], t1[:, 1, :], swt[:, 1, :])

            def v_unit(tci, tt):
                gt = tci * 4 + tt
                ps = pjps.tile([128, HPC * DH], F32, tag="pj", name=f"pjv{gt}")
                for dc in range(DC):
                    nc.tensor.matmul(
                        ps[:],
                        xt[:, dc, tci * TCH + tt * 128 : tci * TCH + (tt + 1) * 128],
                        wv[:, dc, :],
                        start=(dc == 0),
                        stop=(dc == DC - 1),
                    )
                nc.vector.tensor_copy(
                    vaug[:, gt, :, DH:],
                    ps[:].rearrange("p (h d) -> p h d", h=HPC),
                )

            def qk_pair(tci, p):
                return [
                    (lambda: qk_unit(tci, wq, p, 0)),
                    (lambda: qk_unit(tci, wk, p, 1)),
                ]

            def yproj_unit(qc, tt, ni):
                gtt = qc * 4 + tt
                alt = qc == 3 and (tt + ni) % 2 == 1
                pool, tag = (sps, "s") if alt else (pjps, "pj")
                yps = pool.tile([128, TCH], F32, tag=tag, name=f"y{gtt}_{ni}")
                for p2 in range(2):
                    nc.tensor.matmul(
                        yps[:],
                        ot[p2][:, gtt * 128 : (gtt + 1) * 128],
                        wo[:, p2, ni * TCH : (ni + 1) * TCH],
                        start=(p2 == 0),
                        stop=(p2 == 1),
                    )
                ysb = ysbp.tile([128, TCH], F32, tag="ysb", name=f"ys{gtt}_{ni}")
                nc.vector.tensor_copy(ysb[:], yps[:])
                nc.sync.dma_start(
                    y_d.ap()[gtt * 128 : (gtt + 1) * 128, ni * TCH : (ni + 1) * TCH],
                    ysb[:],
                )

            def yproj_units(qc):
                return [
                    (lambda tt=tt, ni=ni: yproj_unit(qc, tt, ni))
                    for tt in range(4)
                    for ni in range(2)
                ]

            OD = 2  # O' trails S by this many k-tiles

            def attn_chunk(qc, units_by_hp):
                nkt = 4 * qc + 4

                for hp in range(2):
                    units = units_by_hp[hp]
                    emitted = [0]
                    it = [0]

                    def fill():
                        it[0] += 1
                        want = len(units) * it[0] // nkt
                        while emitted[0] < want:
                            units[emitted[0]]()
                            emitted[0] += 1

                    h0, h1 = 2 * hp, 2 * hp + 1
                    opsA = ops.tile([128, TCH], F32, tag="o", name=f"oA{qc}_{hp}")
                    opsB = ops.tile([128, TCH], F32, tag="o", name=f"oB{qc}_{hp}")
                    live = {}

                    def s_step(kt):
                        off = max(0, 128 * kt - 512 * qc)
                        sp = sps.tile([128, 2, TCH], F32, tag="s", name=f"s{qc}_{hp}_{kt}")
                        pt = ptp.tile([128, 2, TCH], BF16, tag="pt", name=f"pt{qc}_{hp}_{kt}")
                        live[kt] = (pt, off)
                        for j, h in ((0, h0), (1, h1)):
                            pb = 64 * j
                            nc.tensor.matmul(
                                sp[:, j, off:],
                                ktt[hp][pb : pb + 64, kt * 128 : (kt + 1) * 128],
                                qt[hp][pb : pb + 64, qc * 512 + off : (qc + 1) * 512],
                                start=True,
                                stop=True,
                            )
                        nc.scalar.activation(
                            pt[:, :, off:], sp[:, :, off:], EXP, bias=0.0, scale=0.125
                        )
                        if kt >= 4 * qc:
                            nc.gpsimd.affine_select(
                                out=pt[:, :, off : off + 128],
                                in_=pt[:, :, off : off + 128],
                                compare_op=mybir.AluOpType.is_ge,
                                fill=0.0,
                                base=0,
                                pattern=[[0, 2], [1, 128]],
                                channel_multiplier=-1,
                            )

                    def o_step(kt):
                        pt, off = live.pop(kt)
                        for j, h in ((0, h0), (1, h1)):
                            dst = opsA if j == 0 else opsB
                            nc.tensor.matmul(
                                dst[:, off:],
                                vaug[:, kt, h, :],
                                pt[:, j, off:],
                                start=(kt == 0),
                                stop=(kt == nkt - 1),
                            )

                    for kt in range(nkt):
                        s_step(kt)
                        if kt >= OD:
                            o_step(kt - OD)
                        fill()
                    for kt in range(nkt - OD, nkt):
                        o_step(kt)

                    qsl = slice(qc * 512, (qc + 1) * 512)
                    recA = recp.tile([64, TCH], F32, tag="rec", name=f"rA{qc}_{hp}")
                    nc.vector.reciprocal_approx_fast(recA[:], opsA[0:64, :])
                    nc.vector.tensor_mul(ot[hp][0:64, qsl], opsA[64:128, :], recA[:])
                    recB = recp.tile([64, TCH], F32, tag="rec", name=f"rB{qc}_{hp}")
                    nc.vector.reciprocal_approx_fast(recB[:], opsB[0:64, :])
                    nc.vector.tensor_mul(ot[hp][64:128, qsl], opsB[64:128, :], recB[:])

                    while emitted[0] < len(units):
                        units[emitted[0]]()
                        emitted[0] += 1

            def V(tci, a, b):
                return [(lambda tt=tt: v_unit(tci, tt)) for tt in range(a, b)]

            yp = {qc: yproj_units(qc) for qc in range(4)}

            # minimal dense prologue: just the hp0 pair of chunk 0
            for u in qk_pair(0, 0):
                u()
            attn_chunk(0, [
                V(0, 0, 4) + qk_pair(0, 1),
                qk_pair(1, 0) + qk_pair(1, 1) + V(1, 0, 1),
            ])
            attn_chunk(1, [
                V(1, 1, 4) + qk_pair(2, 0),
                qk_pair(2, 1) + V(2, 0, 2) + yp[0][:4],
            ])
            attn_chunk(2, [
                V(2, 2, 4) + qk_pair(3, 0) + yp[0][4:],
                qk_pair(3, 1) + V(3, 0, 2) + yp[1][:8],
            ])
            attn_chunk(3, [
                V(3, 2, 4) + yp[2][:6],
                yp[2][6:],
            ])
            for u in yp[3]:
                u()
    nc.compile()
    return nc


_NC_CACHE = []


def _get_nc():
    if not _NC_CACHE:
        _NC_CACHE.append(_build())
    return _NC_CACHE[0]


_LAST_RESULTS = []  # stashed BassKernelResults for test harness introspection


def kernel(x, Wqkv, Wout, _trace=False, **_trace_kwargs):
    x = np.asarray(x, dtype=np.float32)
    Wqkv = np.asarray(Wqkv, dtype=np.float32)
    Wout = np.asarray(Wout, dtype=np.float32)

    nc = _get_nc()
    in_maps = []
    for c in range(NCORES):
        b, g = divmod(c, HPC)
        cols = slice(g * HPC * DH, (g + 1) * HPC * DH)
        rows = slice(g * HPC * DH, (g + 1) * HPC * DH)
        bf = ml_dtypes.bfloat16
        def _pc(a, parts=128):  # [C*parts, N] -> [parts, C*N] contiguous
            c = a.shape[0] // parts
            return np.ascontiguousarray(
                a.reshape(c, parts, -1).transpose(1, 0, 2).reshape(parts, -1)
            )

        in_maps.append(
            {
                "xp": _pc(x[b].T.astype(bf)),
                "wq": _pc(Wqkv[:, 0:D][:, cols].astype(bf)),
                "wk": _pc(Wqkv[:, D : 2 * D][:, cols].astype(bf)),
                "wv": _pc(Wqkv[:, 2 * D : 3 * D][:, cols].astype(bf)),
                "wo": _pc(Wout[rows, :].astype(bf)),
            }
        )

    res = run_bass_kernel_spmd(
        nc, in_maps, core_ids=list(range(NCORES)), trace=_trace, **_trace_kwargs
    )
    _LAST_RESULTS.clear()
    _LAST_RESULTS.append(res)

    out = np.zeros((B, T, D), dtype=np.float32)
    for c in range(NCORES):
        b = c // HPC
        out[b] += res.results[c]["y"]
    return out


# revision 4
# speedup vs baseline: 1.0231x; 1.0064x over previous
"""Trainium2 Bass kernel for EnhancedAttention (B=2, T=2048, D=1024, H=16, DH=64).

Sharding: 8 cores = 2 batches x 4 head-groups (4 heads each). No collectives;
each core computes a partial out-projection and the host sums the 4 partials
per batch.

v2 redesign vs baseline:
  - S matmuls for the two heads of a pair are emitted back-to-back with
    base partitions 0/64 so they run CONCURRENTLY in different PE row groups.
  - vaug carries 64 ones-columns so the O' matmul replicates the softmax
    denominator across partitions 64:128; normalization is then just a
    [64,512] DVE reciprocal + [64,512] DVE multiply straight out of PSUM
    (no 1-partition reciprocal, no partition_broadcast, no extract DMA).
  - exp runs only on the causally-live region; the causal mask is applied to
    just the 128-col diagonal block.
  - All dma_starts dispatch from sync/gpsimd so the Scalar engine does
    nothing but exp (it is ~70us of line-rate work, near critical).
  - proj / yproj work units are interleaved into the attention kt-loops so
    the PE never idles while ACT streams exps (keeps HAM at K=8/8).
"""
import os
import sys

for _p in ("/opt/trn_rl_repo", "/root/.axon_site/_ro/trn_rl_repo"):
    if os.path.isdir(_p) and _p not in sys.path:
        sys.path.append(_p)

import ml_dtypes
import numpy as np

import concourse.bass as bass  # noqa: F401
import concourse.tile as tile
from concourse import bacc, mybir
from concourse.bass_utils import run_bass_kernel_spmd

B, T, D = 2, 2048, 1024
H, DH = 16, 64
HPC = 4  # heads per core
NCORES = 8
ROPE_THETA = 10000.0

F32 = mybir.dt.float32
BF16 = mybir.dt.bfloat16
FP16 = mybir.dt.float16

TCH = 512  # t-chunk (q-chunk) size
TC = T // TCH  # 4
DC = D // 128  # 8 contraction chunks
NKT = T // 128  # 16 k-tiles


def _rope_tables():
    inv = 1.0 / (ROPE_THETA ** (np.arange(0, DH, 2, dtype=np.float64) / DH))
    f = np.arange(T, dtype=np.float64)[:, None] * inv[None, :]  # [T, 32]
    cos = np.cos(f).T.astype(np.float16)  # [32, T]
    sin = np.sin(f).T.astype(np.float16)
    cs1 = np.ascontiguousarray(cos)  # [32, T], replicated 4x on chip
    # sign pattern chosen so that after the 32-strip swap the sin terms land
    # as [-sin*x2, sin*x1] against the cos terms
    cs2 = np.ascontiguousarray(np.concatenate([sin, -sin], axis=0))  # [64, T]
    return cs1, cs2


def _build():
    nc = bacc.Bacc("TRN2", target_bir_lowering=False, debug=False, num_devices=NCORES)
    # all inputs pre-arranged on host to [128 partitions, ...] contiguous;
    # x is laid out t-chunk-major so early chunks land first
    x_d = nc.dram_tensor("xp", [4, 128, DC * TCH], BF16, kind="ExternalInput")
    wq_d = nc.dram_tensor("wq", [128, DC * HPC * DH], BF16, kind="ExternalInput")
    wk_d = nc.dram_tensor("wk", [128, DC * HPC * DH], BF16, kind="ExternalInput")
    wv_d = nc.dram_tensor("wv", [128, DC * HPC * DH], BF16, kind="ExternalInput")
    wo_d = nc.dram_tensor("wo", [128, 2 * D], BF16, kind="ExternalInput")
    y_d = nc.dram_tensor("y", [T, D], FP16, kind="ExternalOutput")

    cs1_np, cs2_np = _rope_tables()
    cs1_d = nc.inline_tensor(cs1_np, "cs1")
    cs2_d = nc.inline_tensor(cs2_np, "cs2")

    EXP = mybir.ActivationFunctionType.Exp

    import contextlib
    with tile.TileContext(nc) as tc:
        with (
            contextlib.ExitStack() as _ctx,
            tc.tile_pool(name="sb", bufs=1) as sb,
            tc.tile_pool(name="ropep", bufs=2) as ropep,
            tc.tile_pool(name="ptp", bufs=4) as ptp,
            tc.tile_pool(name="recp", bufs=3) as recp,
            tc.tile_pool(name="ysbp", bufs=3) as ysbp,
        ):
            wq = sb.tile([128, DC, HPC * DH], BF16)
            wk = sb.tile([128, DC, HPC * DH], BF16)
            wv = sb.tile([128, DC, HPC * DH], BF16)
            wo = sb.tile([128, 2, D], BF16)
            cs1 = sb.tile([128, T], FP16)
            cs2 = sb.tile([128, T], FP16)
            qt = [sb.tile([128, T], BF16, tag=f"qt{p}", name=f"qt{p}") for p in range(2)]
            ktt = [sb.tile([128, T], BF16, tag=f"kt{p}", name=f"kt{p}") for p in range(2)]
            # per (ktile, head): [ones (64) | V (64 dh)] -> O' matmul yields
            # softmax denominator on partitions 0:64 (base-0 for the custom
            # fast-reciprocal op), attention numerator on 64:128
            vaug = sb.tile([128, NKT, HPC, 2 * DH], BF16)
            ot = [sb.tile([128, T], BF16, tag=f"ot{p}", name=f"ot{p}") for p in range(2)]

            # whole x resident in SBUF; all DRAM reads are contiguous.
            # spread the 7MB of input loads across 4 HWDGE queues; gpsimd
            # (slow SWDGE) only gets wo which isn't needed until ~60us.
            # queue roles: scalar = bulk x (t-chunk-major) + wk/wv;
            # sync = wq/cs then latency-critical swaps + y writes;
            # gpsimd = wo/memset/affines only.
            xt = sb.tile([128, DC, T], BF16, name="xt")
            nc.sync.dma_start(wq[:], wq_d.ap().rearrange("p (c n) -> p c n", c=DC))
            nc.scalar.dma_start(wk[:], wk_d.ap().rearrange("p (c n) -> p c n", c=DC))
            nc.scalar.dma_start(
                xt[:, :, 0:TCH], x_d.ap()[0].rearrange("p (c t) -> p c t", c=DC)
            )
            nc.sync.dma_start(cs1[0:32, :], cs1_d.ap())
            nc.sync.dma_start(cs2[0:64, :], cs2_d.ap())
            nc.sync.dma_start(cs1[32:64, :], cs1[0:32, :])
            nc.sync.dma_start(cs1[64:128, :], cs1[0:64, :])
            nc.sync.dma_start(cs2[64:128, :], cs2[0:64, :])
            nc.scalar.dma_start(wv[:], wv_d.ap().rearrange("p (c n) -> p c n", c=DC))
            for tk in range(1, 4):
                nc.scalar.dma_start(
                    xt[:, :, tk * TCH : (tk + 1) * TCH],
                    x_d.ap()[tk].rearrange("p (c t) -> p c t", c=DC),
                )
            nc.gpsimd.dma_start(wo[:], wo_d.ap().rearrange("p (c n) -> p c n", c=2))
            nc.gpsimd.memset(vaug[:, :, :, 0:DH], 1.0)

            pjps = _ctx.enter_context(tc.tile_pool(name="pjps", bufs=2, space="PSUM"))
            sps = _ctx.enter_context(tc.tile_pool(name="sps", bufs=2, space="PSUM"))
            ops = _ctx.enter_context(tc.tile_pool(name="ops", bufs=2, space="PSUM"))

            # PE warm-up: fills the input-DMA wait so HAM reaches K=8/8
            warm = sb.tile([128, TCH], BF16, name="warm")
            nc.vector.memset(warm, 0.0)
            wps = pjps.tile([128, TCH], F32, tag="pj", name="wps")
            for wi in range(20):
                nc.tensor.matmul(
                    wps[:, 0:256], warm[:, 0:128], warm[:, 0:256],
                    start=(wi == 0), stop=(wi == 19),
                )

            rope_bufs = {}

            def qk_unit(tci, w_sb, p, wi):
                # wi=0: Q projection (allocates the pair's rope tiles);
                # wi=1: K projection + strip-swap + adds -> qt[p]/ktt[p] ready
                tsl = slice(tci * TCH, (tci + 1) * TCH)
                ps = pjps.tile([128, TCH], F32, tag="pj", name=f"pj{tci}_{p}_{wi}")
                for dc in range(DC):
                    nc.tensor.matmul(
                        ps[:],
                        w_sb[:, dc, p * 128 : (p + 1) * 128],
                        xt[:, dc, tsl],
                        start=(dc == 0),
                        stop=(dc == DC - 1),
                    )
                if wi == 0:
                    t1 = ropep.tile([128, 2, TCH], FP16, tag="t1", name=f"t1_{tci}_{p}")
                    t2 = ropep.tile([128, 2, TCH], FP16, tag="t2", name=f"t2_{tci}_{p}")
                    swt = ropep.tile([128, 2, TCH], FP16, tag="swt", name=f"sw_{tci}_{p}")
                    rope_bufs[(tci, p)] = (t1, t2, swt)
                else:
                    t1, t2, swt = rope_bufs[(tci, p)]
                # fast PSUM evac (frees the pjps slot), then 16-bit DVE math
                pe = ropep.tile([128, TCH], FP16, tag="pe", name=f"pe{tci}_{p}_{wi}")
                nc.vector.tensor_copy(pe[:], ps[:])
                nc.vector.tensor_mul(t1[:, wi, :], pe[:], cs1[:, tsl])
                nc.vector.tensor_mul(t2[:, wi, :], pe[:], cs2[:, tsl])
                if wi == 1:
                    # 32-strip swap (0<->1, 2<->3) across the (q, k) pair
                    for s in range(4):
                        nc.sync.dma_start(
                            swt[s * 32 : (s + 1) * 32, :, :],
                            t2[(s ^ 1) * 32 : ((s ^ 1) + 1) * 32, :, :],
                        )
                    nc.vector.tensor_add(qt[p][:, tsl], t1[:, 0, :], swt[:, 0, :])
                    nc.vector.tensor_add(ktt[p][:, tsl], t1[:, 1, :], swt[:, 1, :])

            def v_unit(tci, tt):
                gt = tci * 4 + tt
                ps = pjps.tile([128, HPC * DH], F32, tag="pj", name=f"pjv{gt}")
                for dc in range(DC):
                    nc.tensor.matmul(
                        ps[:],
                        xt[:, dc, tci * TCH + tt * 128 : tci * TCH + (tt + 1) * 128],
                        wv[:, dc, :],
                        start=(dc == 0),
                        stop=(dc == DC - 1),
                    )
                nc.vector.tensor_copy(
                    vaug[:, gt, :, DH:],
                    ps[:].rearrange("p (h d) -> p h d", h=HPC),
                )

            def qk_pair(tci, p):
                return [
                    (lambda: qk_unit(tci, wq, p, 0)),
                    (lambda: qk_unit(tci, wk, p, 1)),
                ]

            def yproj_unit(qc, tt, ni):
                gtt = qc * 4 + tt
                alt = qc == 3 and (tt + ni) % 2 == 1
                pool, tag = (sps, "s") if alt else (pjps, "pj")
                yps = pool.tile([128, TCH], F32, tag=tag, name=f"y{gtt}_{ni}")
                for p2 in range(2):
                    nc.tensor.matmul(
                        yps[:],
                        ot[p2][:, gtt * 128 : (gtt + 1) * 128],
                        wo[:, p2, ni * TCH : (ni + 1) * TCH],
                        start=(p2 == 0),
                        stop=(p2 == 1),
                    )
                ysb = ysbp.tile([128, TCH], FP16, tag="ysb", name=f"ys{gtt}_{ni}")
                if alt:
                    nc.scalar.activation(
                        ysb[:], yps[:], mybir.ActivationFunctionType.Copy, bias=0.0
                    )
                else:
                    nc.vector.tensor_copy(ysb[:], yps[:])
                nc.sync.dma_start(
                    y_d.ap()[gtt * 128 : (gtt + 1) * 128, ni * TCH : (ni + 1) * TCH],
                    ysb[:],
                )

            def yproj_units(qc):
                return [
                    (lambda tt=tt, ni=ni: yproj_unit(qc, tt, ni))
                    for tt in range(4)
                    for ni in range(2)
                ]

            OD = 2  # O' trails S by this many k-tiles

            def attn_chunk(qc, units_by_hp):
                nkt = 4 * qc + 4

                for hp in range(2):
                    units = units_by_hp[hp]
                    emitted = [0]
                    it = [0]

                    def fill():
                        it[0] += 1
                        want = len(units) * it[0] // nkt
                        while emitted[0] < want:
                            units[emitted[0]]()
                            emitted[0] += 1

                    h0, h1 = 2 * hp, 2 * hp + 1
                    opsA = ops.tile([128, TCH], F32, tag="o", name=f"oA{qc}_{hp}")
                    opsB = ops.tile([128, TCH], F32, tag="o", name=f"oB{qc}_{hp}")
                    live = {}

                    def s_step(kt):
                        off = max(0, 128 * kt - 512 * qc)
                        sp = sps.tile([128, 2, TCH], F32, tag="s", name=f"s{qc}_{hp}_{kt}")
                        pt = ptp.tile([128, 2, TCH], BF16, tag="pt", name=f"pt{qc}_{hp}_{kt}")
                        live[kt] = (pt, off)
                        for j, h in ((0, h0), (1, h1)):
                            pb = 64 * j
                            nc.tensor.matmul(
                                sp[:, j, off:],
                                ktt[hp][pb : pb + 64, kt * 128 : (kt + 1) * 128],
                                qt[hp][pb : pb + 64, qc * 512 + off : (qc + 1) * 512],
                                start=True,
                                stop=True,
                            )
                        nc.scalar.activation(
                            pt[:, :, off:], sp[:, :, off:], EXP, bias=0.0, scale=0.125
                        )
                        if kt >= 4 * qc:
                            nc.gpsimd.affine_select(
                                out=pt[:, :, off : off + 128],
                                in_=pt[:, :, off : off + 128],
                                compare_op=mybir.AluOpType.is_ge,
                                fill=0.0,
                                base=0,
                                pattern=[[0, 2], [1, 128]],
                                channel_multiplier=-1,
                            )

                    def o_step(kt):
                        pt, off = live.pop(kt)
                        for j, h in ((0, h0), (1, h1)):
                            dst = opsA if j == 0 else opsB
                            nc.tensor.matmul(
                                dst[:, off:],
                                vaug[:, kt, h, :],
                                pt[:, j, off:],
                                start=(kt == 0),
                                stop=(kt == nkt - 1),
                            )

                    for kt in range(nkt):
                        s_step(kt)
                        if kt >= OD:
                            o_step(kt - OD)
                        fill()
                    for kt in range(nkt - OD, nkt):
                        o_step(kt)

                    qsl = slice(qc * 512, (qc + 1) * 512)
                    recA = recp.tile([64, TCH], F32, tag="rec", name=f"rA{qc}_{hp}")
                    nc.vector.reciprocal_approx_fast(recA[:], opsA[0:64, :])
                    nc.vector.tensor_mul(ot[hp][0:64, qsl], opsA[64:128, :], recA[:])
                    recB = recp.tile([64, TCH], F32, tag="rec", name=f"rB{qc}_{hp}")
                    nc.vector.reciprocal_approx_fast(recB[:], opsB[0:64, :])
                    nc.vector.tensor_mul(ot[hp][64:128, qsl], opsB[64:128, :], recB[:])

                    while emitted[0] < len(units):
                        units[emitted[0]]()
                        emitted[0] += 1

            def V(tci, a, b):
                return [(lambda tt=tt: v_unit(tci, tt)) for tt in range(a, b)]

            yp = {qc: yproj_units(qc) for qc in range(4)}

            # minimal dense prologue: just the hp0 pair of chunk 0
            for u in qk_pair(0, 0):
                u()
            attn_chunk(0, [
                V(0, 0, 4) + qk_pair(0, 1),
                qk_pair(1, 0) + qk_pair(1, 1) + V(1, 0, 1),
            ])
            attn_chunk(1, [
                V(1, 1, 4) + qk_pair(2, 0),
                qk_pair(2, 1) + V(2, 0, 2) + yp[0][:4],
            ])
            attn_chunk(2, [
                V(2, 2, 4) + qk_pair(3, 0) + yp[0][4:],
                qk_pair(3, 1) + V(3, 0, 2) + yp[1][:8],
            ])
            attn_chunk(3, [
                V(3, 2, 4) + yp[2][:6],
                yp[2][6:],
            ])
            for u in yp[3]:
                u()
    nc.compile()
    return nc


_NC_CACHE = []


def _get_nc():
    if not _NC_CACHE:
        _NC_CACHE.append(_build())
    return _NC_CACHE[0]


_LAST_RESULTS = []  # stashed BassKernelResults for test harness introspection


def kernel(x, Wqkv, Wout, _trace=False, **_trace_kwargs):
    x = np.asarray(x, dtype=np.float32)
    Wqkv = np.asarray(Wqkv, dtype=np.float32)
    Wout = np.asarray(Wout, dtype=np.float32)

    nc = _get_nc()
    in_maps = []
    for c in range(NCORES):
        b, g = divmod(c, HPC)
        cols = slice(g * HPC * DH, (g + 1) * HPC * DH)
        rows = slice(g * HPC * DH, (g + 1) * HPC * DH)
        bf = ml_dtypes.bfloat16
        def _pc(a, parts=128):  # [C*parts, N] -> [parts, C*N] contiguous
            c = a.shape[0] // parts
            return np.ascontiguousarray(
                a.reshape(c, parts, -1).transpose(1, 0, 2).reshape(parts, -1)
            )

        in_maps.append(
            {
                "xp": np.ascontiguousarray(
                    x[b].T.astype(bf)
                    .reshape(DC, 128, 4, TCH)
                    .transpose(2, 1, 0, 3)
                    .reshape(4, 128, DC * TCH)
                ),
                "wq": _pc(Wqkv[:, 0:D][:, cols].astype(bf)),
                "wk": _pc(Wqkv[:, D : 2 * D][:, cols].astype(bf)),
                "wv": _pc(Wqkv[:, 2 * D : 3 * D][:, cols].astype(bf)),
                "wo": _pc(Wout[rows, :].astype(bf)),
            }
        )

    res = run_bass_kernel_spmd(
        nc, in_maps, core_ids=list(range(NCORES)), trace=_trace, **_trace_kwargs
    )
    _LAST_RESULTS.clear()
    _LAST_RESULTS.append(res)

    out = np.zeros((B, T, D), dtype=np.float32)
    for c in range(NCORES):
        b = c // HPC
        out[b] += res.results[c]["y"].astype(np.float32)
    return out


# revision 5
# speedup vs baseline: 1.0477x; 1.0240x over previous
"""Trainium2 Bass kernel for EnhancedAttention (B=2, T=2048, D=1024, H=16, DH=64).

Sharding: 8 cores = 2 batches x 4 head-groups (4 heads each). No collectives;
each core computes a partial out-projection and the host sums the 4 partials
per batch.

v2 redesign vs baseline:
  - S matmuls for the two heads of a pair are emitted back-to-back with
    base partitions 0/64 so they run CONCURRENTLY in different PE row groups.
  - vaug carries 64 ones-columns so the O' matmul replicates the softmax
    denominator across partitions 64:128; normalization is then just a
    [64,512] DVE reciprocal + [64,512] DVE multiply straight out of PSUM
    (no 1-partition reciprocal, no partition_broadcast, no extract DMA).
  - exp runs only on the causally-live region; the causal mask is applied to
    just the 128-col diagonal block.
  - All dma_starts dispatch from sync/gpsimd so the Scalar engine does
    nothing but exp (it is ~70us of line-rate work, near critical).
  - proj / yproj work units are interleaved into the attention kt-loops so
    the PE never idles while ACT streams exps (keeps HAM at K=8/8).
"""
import os
import sys

for _p in ("/opt/trn_rl_repo", "/root/.axon_site/_ro/trn_rl_repo"):
    if os.path.isdir(_p) and _p not in sys.path:
        sys.path.append(_p)

import ml_dtypes
import numpy as np

import concourse.bass as bass  # noqa: F401
import concourse.tile as tile
from concourse import bacc, mybir
from concourse.bass_utils import run_bass_kernel_spmd

B, T, D = 2, 2048, 1024
H, DH = 16, 64
HPC = 4  # heads per core
NCORES = 8
ROPE_THETA = 10000.0

F32 = mybir.dt.float32
BF16 = mybir.dt.bfloat16
FP16 = mybir.dt.float16

TCH = 512  # t-chunk (q-chunk) size
TC = T // TCH  # 4
DC = D // 128  # 8 contraction chunks
NKT = T // 128  # 16 k-tiles


def _rope_tables():
    inv = 1.0 / (ROPE_THETA ** (np.arange(0, DH, 2, dtype=np.float64) / DH))
    f = np.arange(T, dtype=np.float64)[:, None] * inv[None, :]  # [T, 32]
    cos = np.cos(f).T.astype(np.float16)  # [32, T]
    sin = np.sin(f).T.astype(np.float16)
    cs1 = np.ascontiguousarray(cos)  # [32, T], replicated 4x on chip
    # sign pattern chosen so that after the 32-strip swap the sin terms land
    # as [-sin*x2, sin*x1] against the cos terms
    cs2 = np.ascontiguousarray(np.concatenate([sin, -sin], axis=0))  # [64, T]
    return cs1, cs2


def _build():
    nc = bacc.Bacc("TRN2", target_bir_lowering=False, debug=False, num_devices=NCORES)
    # all inputs pre-arranged on host to [128 partitions, ...] contiguous;
    # x is laid out t-chunk-major so early chunks land first
    x_d = nc.dram_tensor("xp", [4, 128, DC * TCH], BF16, kind="ExternalInput")
    wq_d = nc.dram_tensor("wq", [128, DC * HPC * DH], BF16, kind="ExternalInput")
    wk_d = nc.dram_tensor("wk", [128, DC * HPC * DH], BF16, kind="ExternalInput")
    wv_d = nc.dram_tensor("wv", [128, DC * HPC * DH], BF16, kind="ExternalInput")
    wo_d = nc.dram_tensor("wo", [128, 2 * D], BF16, kind="ExternalInput")
    y_d = nc.dram_tensor("y", [T, D], FP16, kind="ExternalOutput")

    cs1_np, cs2_np = _rope_tables()
    cs1_d = nc.inline_tensor(cs1_np, "cs1")
    cs2_d = nc.inline_tensor(cs2_np, "cs2")

    EXP = mybir.ActivationFunctionType.Exp

    import contextlib
    with tile.TileContext(nc) as tc:
        with (
            contextlib.ExitStack() as _ctx,
            tc.tile_pool(name="sb", bufs=1) as sb,
            tc.tile_pool(name="ropep", bufs=2) as ropep,
            tc.tile_pool(name="ptp", bufs=6) as ptp,
            tc.tile_pool(name="recp", bufs=3) as recp,
            tc.tile_pool(name="ysbp", bufs=3) as ysbp,
        ):
            wq = sb.tile([128, DC, HPC * DH], BF16)
            wk = sb.tile([128, DC, HPC * DH], BF16)
            wv = sb.tile([128, DC, HPC * DH], BF16)
            wo = sb.tile([128, 2, D], BF16)
            cs1 = sb.tile([128, T], FP16)
            cs2 = sb.tile([128, T], FP16)
            qt = [sb.tile([128, T], BF16, tag=f"qt{p}", name=f"qt{p}") for p in range(2)]
            ktt = [sb.tile([128, T], BF16, tag=f"kt{p}", name=f"kt{p}") for p in range(2)]
            # per (ktile, head): [ones (64) | V (64 dh)] -> O' matmul yields
            # softmax denominator on partitions 0:64 (base-0 for the custom
            # fast-reciprocal op), attention numerator on 64:128
            vaug = sb.tile([128, NKT, HPC, 2 * DH], BF16)
            ot = [sb.tile([128, T], BF16, tag=f"ot{p}", name=f"ot{p}") for p in range(2)]

            # whole x resident in SBUF; all DRAM reads are contiguous.
            # spread the 7MB of input loads across 4 HWDGE queues; gpsimd
            # (slow SWDGE) only gets wo which isn't needed until ~60us.
            # queue roles: scalar = bulk x (t-chunk-major) + wk/wv;
            # sync = wq/cs then latency-critical swaps + y writes;
            # gpsimd = wo/memset/affines only.
            xt = sb.tile([128, DC, T], BF16, name="xt")
            x0_r = x_d.ap()[0].rearrange("p (c t) -> p c t", c=DC)
            nc.sync.dma_start(xt[:, 0:4, 0:TCH], x0_r[:, 0:4, :])
            nc.scalar.dma_start(xt[:, 4:8, 0:TCH], x0_r[:, 4:8, :])
            nc.sync.dma_start(wq[:], wq_d.ap().rearrange("p (c n) -> p c n", c=DC))
            nc.scalar.dma_start(wk[:], wk_d.ap().rearrange("p (c n) -> p c n", c=DC))
            nc.sync.dma_start(cs1[0:32, :], cs1_d.ap())
            nc.sync.dma_start(cs2[0:64, :], cs2_d.ap())
            nc.sync.dma_start(cs1[32:64, :], cs1[0:32, :])
            nc.sync.dma_start(cs1[64:128, :], cs1[0:64, :])
            nc.sync.dma_start(cs2[64:128, :], cs2[0:64, :])
            nc.scalar.dma_start(
                xt[:, :, TCH : 2 * TCH], x_d.ap()[1].rearrange("p (c t) -> p c t", c=DC)
            )
            nc.scalar.dma_start(wv[:], wv_d.ap().rearrange("p (c n) -> p c n", c=DC))
            for tk in range(2, 4):
                nc.scalar.dma_start(
                    xt[:, :, tk * TCH : (tk + 1) * TCH],
                    x_d.ap()[tk].rearrange("p (c t) -> p c t", c=DC),
                )
            nc.gpsimd.dma_start(wo[:], wo_d.ap().rearrange("p (c n) -> p c n", c=2))
            nc.gpsimd.memset(vaug[:, :, :, 0:DH], 1.0)

            pjps = _ctx.enter_context(tc.tile_pool(name="pjps", bufs=2, space="PSUM"))
            sps = _ctx.enter_context(tc.tile_pool(name="sps", bufs=2, space="PSUM"))
            ops = _ctx.enter_context(tc.tile_pool(name="ops", bufs=2, space="PSUM"))

            # PE warm-up: fills the input-DMA wait so HAM reaches K=8/8
            warm = sb.tile([128, TCH], BF16, name="warm")
            nc.vector.memset(warm, 0.0)
            wps = pjps.tile([128, TCH], F32, tag="pj", name="wps")
            for wi in range(32):
                nc.tensor.matmul(
                    wps[:, 0:256], warm[:, 0:128], warm[:, 0:256],
                    start=(wi == 0), stop=(wi == 31),
                )

            rope_bufs = {}

            def qk_unit(tci, w_sb, p, wi):
                # wi=0: Q projection (allocates the pair's rope tiles);
                # wi=1: K projection + strip-swap + adds -> qt[p]/ktt[p] ready
                tsl = slice(tci * TCH, (tci + 1) * TCH)
                ps = pjps.tile([128, TCH], F32, tag="pj", name=f"pj{tci}_{p}_{wi}")
                for dc in range(DC):
                    nc.tensor.matmul(
                        ps[:],
                        w_sb[:, dc, p * 128 : (p + 1) * 128],
                        xt[:, dc, tsl],
                        start=(dc == 0),
                        stop=(dc == DC - 1),
                    )
                if wi == 0:
                    t1 = ropep.tile([128, 2, TCH], FP16, tag="t1", name=f"t1_{tci}_{p}")
                    t2 = ropep.tile([128, 2, TCH], FP16, tag="t2", name=f"t2_{tci}_{p}")
                    swt = ropep.tile([128, 2, TCH], FP16, tag="swt", name=f"sw_{tci}_{p}")
                    rope_bufs[(tci, p)] = (t1, t2, swt)
                else:
                    t1, t2, swt = rope_bufs[(tci, p)]
                # fast PSUM evac (frees the pjps slot), then 16-bit DVE math
                pe = ropep.tile([128, TCH], FP16, tag="pe", name=f"pe{tci}_{p}_{wi}")
                nc.vector.tensor_copy(pe[:], ps[:])
                nc.vector.tensor_mul(t1[:, wi, :], pe[:], cs1[:, tsl])
                nc.vector.tensor_mul(t2[:, wi, :], pe[:], cs2[:, tsl])
                if wi == 1:
                    # 32-strip swap (0<->1, 2<->3) across the (q, k) pair
                    for s in range(4):
                        nc.sync.dma_start(
                            swt[s * 32 : (s + 1) * 32, :, :],
                            t2[(s ^ 1) * 32 : ((s ^ 1) + 1) * 32, :, :],
                        )
                    nc.vector.tensor_add(qt[p][:, tsl], t1[:, 0, :], swt[:, 0, :])
                    nc.vector.tensor_add(ktt[p][:, tsl], t1[:, 1, :], swt[:, 1, :])

            def v_unit(tci, tt):
                gt = tci * 4 + tt
                ps = pjps.tile([128, HPC * DH], F32, tag="pj", name=f"pjv{gt}")
                for dc in range(DC):
                    nc.tensor.matmul(
                        ps[:],
                        xt[:, dc, tci * TCH + tt * 128 : tci * TCH + (tt + 1) * 128],
                        wv[:, dc, :],
                        start=(dc == 0),
                        stop=(dc == DC - 1),
                    )
                nc.vector.tensor_copy(
                    vaug[:, gt, :, DH:],
                    ps[:].rearrange("p (h d) -> p h d", h=HPC),
                )

            def qk_pair(tci, p):
                return [
                    (lambda: qk_unit(tci, wq, p, 0)),
                    (lambda: qk_unit(tci, wk, p, 1)),
                ]

            def yproj_unit(qc, tt, ni):
                gtt = qc * 4 + tt
                alt = qc == 3 and (tt + ni) % 2 == 1
                pool, tag = (sps, "s") if alt else (pjps, "pj")
                yps = pool.tile([128, TCH], F32, tag=tag, name=f"y{gtt}_{ni}")
                for p2 in range(2):
                    nc.tensor.matmul(
                        yps[:],
                        ot[p2][:, gtt * 128 : (gtt + 1) * 128],
                        wo[:, p2, ni * TCH : (ni + 1) * TCH],
                        start=(p2 == 0),
                        stop=(p2 == 1),
                    )
                ysb = ysbp.tile([128, TCH], FP16, tag="ysb", name=f"ys{gtt}_{ni}")
                if alt:
                    nc.scalar.activation(
                        ysb[:], yps[:], mybir.ActivationFunctionType.Copy, bias=0.0
                    )
                else:
                    nc.vector.tensor_copy(ysb[:], yps[:])
                nc.sync.dma_start(
                    y_d.ap()[gtt * 128 : (gtt + 1) * 128, ni * TCH : (ni + 1) * TCH],
                    ysb[:],
                )

            def yproj_units(qc):
                return [
                    (lambda tt=tt, ni=ni: yproj_unit(qc, tt, ni))
                    for tt in range(4)
                    for ni in range(2)
                ]

            OD = 3  # O' trails S by this many k-tiles
            pending = []  # deferred O'-tail + norm from the previous hp stream

            def attn_chunk(qc, units_by_hp):
                nkt = 4 * qc + 4

                for hp in range(2):
                    units = units_by_hp[hp]
                    emitted = [0]
                    it = [0]

                    def fill():
                        it[0] += 1
                        want = len(units) * it[0] // nkt
                        while emitted[0] < want:
                            units[emitted[0]]()
                            emitted[0] += 1

                    h0, h1 = 2 * hp, 2 * hp + 1
                    opsA = ops.tile([128, TCH], F32, tag="o", name=f"oA{qc}_{hp}")
                    opsB = ops.tile([128, TCH], F32, tag="o", name=f"oB{qc}_{hp}")
                    live = {}

                    def s_step(kt):
                        off = max(0, 128 * kt - 512 * qc)
                        sp = sps.tile([128, 2, TCH], F32, tag="s", name=f"s{qc}_{hp}_{kt}")
                        pt = ptp.tile([128, 2, TCH], BF16, tag="pt", name=f"pt{qc}_{hp}_{kt}")
                        live[kt] = (pt, off)
                        for j, h in ((0, h0), (1, h1)):
                            pb = 64 * j
                            nc.tensor.matmul(
                                sp[:, j, off:],
                                ktt[hp][pb : pb + 64, kt * 128 : (kt + 1) * 128],
                                qt[hp][pb : pb + 64, qc * 512 + off : (qc + 1) * 512],
                                start=True,
                                stop=True,
                            )
                        nc.scalar.activation(
                            pt[:, :, off:], sp[:, :, off:], EXP, bias=0.0, scale=0.125
                        )
                        if kt >= 4 * qc:
                            nc.gpsimd.affine_select(
                                out=pt[:, :, off : off + 128],
                                in_=pt[:, :, off : off + 128],
                                compare_op=mybir.AluOpType.is_ge,
                                fill=0.0,
                                base=0,
                                pattern=[[0, 2], [1, 128]],
                                channel_multiplier=-1,
                            )

                    def o_step(kt, live=live, opsA=opsA, opsB=opsB,
                               h0=h0, h1=h1, nkt=nkt):
                        pt, off = live.pop(kt)
                        for j, h in ((0, h0), (1, h1)):
                            dst = opsA if j == 0 else opsB
                            nc.tensor.matmul(
                                dst[:, off:],
                                vaug[:, kt, h, :],
                                pt[:, j, off:],
                                start=(kt == 0),
                                stop=(kt == nkt - 1),
                            )

                    def norm_step(opsA=None, opsB=None, hp=hp, qc=qc):
                        qsl = slice(qc * 512, (qc + 1) * 512)
                        recA = recp.tile([64, TCH], F32, tag="rec", name=f"rA{qc}_{hp}")
                        nc.vector.reciprocal_approx_fast(recA[:], opsA[0:64, :])
                        nc.vector.tensor_mul(
                            ot[hp][0:64, qsl], opsA[64:128, :], recA[:]
                        )
                        recB = recp.tile([64, TCH], F32, tag="rec", name=f"rB{qc}_{hp}")
                        nc.vector.reciprocal_approx_fast(recB[:], opsB[0:64, :])
                        nc.vector.tensor_mul(
                            ot[hp][64:128, qsl], opsB[64:128, :], recB[:]
                        )

                    for kt in range(nkt):
                        s_step(kt)
                        if pending:
                            pending.pop(0)()
                        if kt >= OD:
                            o_step(kt - OD)
                        fill()
                    while pending:
                        pending.pop(0)()
                    pending.extend(
                        [
                            (lambda kt=kt, os=o_step: os(kt))
                            for kt in range(nkt - OD, nkt)
                        ]
                        + [
                            (lambda ns=norm_step, a=opsA, b=opsB: ns(
                                opsA=a, opsB=b
                            )),
                        ]
                    )

                    while emitted[0] < len(units):
                        units[emitted[0]]()
                        emitted[0] += 1

            def V(tci, a, b):
                return [(lambda tt=tt: v_unit(tci, tt)) for tt in range(a, b)]

            yp = {qc: yproj_units(qc) for qc in range(4)}

            # minimal dense prologue: just the hp0 pair of chunk 0
            for u in qk_pair(0, 0):
                u()
            attn_chunk(0, [
                V(0, 0, 4) + qk_pair(0, 1),
                qk_pair(1, 0) + qk_pair(1, 1) + V(1, 0, 1),
            ])
            attn_chunk(1, [
                V(1, 1, 4) + qk_pair(2, 0),
                qk_pair(2, 1) + V(2, 0, 2) + yp[0][:4],
            ])
            attn_chunk(2, [
                V(2, 2, 4) + qk_pair(3, 0) + yp[0][4:],
                qk_pair(3, 1) + V(3, 0, 2) + yp[1][:8],
            ])
            attn_chunk(3, [
                V(3, 2, 4) + yp[2][:6],
                yp[2][6:],
            ])
            while pending:
                pending.pop(0)()
            for u in yp[3]:
                u()
    nc.compile()
    return nc


_NC_CACHE = []


def _get_nc():
    if not _NC_CACHE:
        _NC_CACHE.append(_build())
    return _NC_CACHE[0]


_LAST_RESULTS = []  # stashed BassKernelResults for test harness introspection


def kernel(x, Wqkv, Wout, _trace=False, **_trace_kwargs):
    x = np.asarray(x, dtype=np.float32)
    Wqkv = np.asarray(Wqkv, dtype=np.float32)
    Wout = np.asarray(Wout, dtype=np.float32)

    nc = _get_nc()
    in_maps = []
    for c in range(NCORES):
        b, g = divmod(c, HPC)
        cols = slice(g * HPC * DH, (g + 1) * HPC * DH)
        rows = slice(g * HPC * DH, (g + 1) * HPC * DH)
        bf = ml_dtypes.bfloat16
        def _pc(a, parts=128):  # [C*parts, N] -> [parts, C*N] contiguous
            c = a.shape[0] // parts
            return np.ascontiguousarray(
                a.reshape(c, parts, -1).transpose(1, 0, 2).reshape(parts, -1)
            )

        in_maps.append(
            {
                "xp": np.ascontiguousarray(
                    x[b].T.astype(bf)
                    .reshape(DC, 128, 4, TCH)
                    .transpose(2, 1, 0, 3)
                    .reshape(4, 128, DC * TCH)
                ),
                "wq": _pc(Wqkv[:, 0:D][:, cols].astype(bf)),
                "wk": _pc(Wqkv[:, D : 2 * D][:, cols].astype(bf)),
                "wv": _pc(Wqkv[:, 2 * D : 3 * D][:, cols].astype(bf)),
                "wo": _pc(Wout[rows, :].astype(bf)),
            }
        )

    res = run_bass_kernel_spmd(
        nc, in_maps, core_ids=list(range(NCORES)), trace=_trace, **_trace_kwargs
    )
    _LAST_RESULTS.clear()
    _LAST_RESULTS.append(res)

    out = np.zeros((B, T, D), dtype=np.float32)
    for c in range(NCORES):
        b = c // HPC
        out[b] += res.results[c]["y"].astype(np.float32)
    return out


# revision 6
# speedup vs baseline: 1.0499x; 1.0021x over previous
"""Trainium2 Bass kernel for EnhancedAttention (B=2, T=2048, D=1024, H=16, DH=64).

Sharding: 8 cores = 2 batches x 4 head-groups (4 heads each). No collectives;
each core computes a partial out-projection and the host sums the 4 partials
per batch.

v2 redesign vs baseline:
  - S matmuls for the two heads of a pair are emitted back-to-back with
    base partitions 0/64 so they run CONCURRENTLY in different PE row groups.
  - vaug carries 64 ones-columns so the O' matmul replicates the softmax
    denominator across partitions 64:128; normalization is then just a
    [64,512] DVE reciprocal + [64,512] DVE multiply straight out of PSUM
    (no 1-partition reciprocal, no partition_broadcast, no extract DMA).
  - exp runs only on the causally-live region; the causal mask is applied to
    just the 128-col diagonal block.
  - All dma_starts dispatch from sync/gpsimd so the Scalar engine does
    nothing but exp (it is ~70us of line-rate work, near critical).
  - proj / yproj work units are interleaved into the attention kt-loops so
    the PE never idles while ACT streams exps (keeps HAM at K=8/8).
"""
import os
import sys

for _p in ("/opt/trn_rl_repo", "/root/.axon_site/_ro/trn_rl_repo"):
    if os.path.isdir(_p) and _p not in sys.path:
        sys.path.append(_p)

import ml_dtypes
import numpy as np

import concourse.bass as bass  # noqa: F401
import concourse.tile as tile
from concourse import bacc, mybir
from concourse.bass_utils import run_bass_kernel_spmd

B, T, D = 2, 2048, 1024
H, DH = 16, 64
HPC = 4  # heads per core
NCORES = 8
ROPE_THETA = 10000.0

F32 = mybir.dt.float32
BF16 = mybir.dt.bfloat16
FP16 = mybir.dt.float16

TCH = 512  # t-chunk (q-chunk) size
TC = T // TCH  # 4
DC = D // 128  # 8 contraction chunks
NKT = T // 128  # 16 k-tiles


def _rope_tables():
    inv = 1.0 / (ROPE_THETA ** (np.arange(0, DH, 2, dtype=np.float64) / DH))
    f = np.arange(T, dtype=np.float64)[:, None] * inv[None, :]  # [T, 32]
    cos = np.cos(f).T.astype(np.float16)  # [32, T]
    sin = np.sin(f).T.astype(np.float16)
    cs1 = np.ascontiguousarray(cos)  # [32, T], replicated 4x on chip
    # sign pattern chosen so that after the 32-strip swap the sin terms land
    # as [-sin*x2, sin*x1] against the cos terms
    cs2 = np.ascontiguousarray(np.concatenate([sin, -sin], axis=0))  # [64, T]
    return cs1, cs2


def _build():
    nc = bacc.Bacc("TRN2", target_bir_lowering=False, debug=False, num_devices=NCORES)
    # all inputs pre-arranged on host to [128 partitions, ...] contiguous;
    # x is laid out t-chunk-major so early chunks land first
    x_d = nc.dram_tensor("xp", [4, 128, DC * TCH], BF16, kind="ExternalInput")
    wq_d = nc.dram_tensor("wq", [128, DC * HPC * DH], BF16, kind="ExternalInput")
    wk_d = nc.dram_tensor("wk", [128, DC * HPC * DH], BF16, kind="ExternalInput")
    wv_d = nc.dram_tensor("wv", [128, DC * HPC * DH], BF16, kind="ExternalInput")
    wo_d = nc.dram_tensor("wo", [128, 2 * D], BF16, kind="ExternalInput")
    y_d = nc.dram_tensor("y", [T, D], FP16, kind="ExternalOutput")

    cs1_np, cs2_np = _rope_tables()
    cs1_d = nc.inline_tensor(cs1_np, "cs1")
    cs2_d = nc.inline_tensor(cs2_np, "cs2")

    EXP = mybir.ActivationFunctionType.Exp

    import contextlib
    with tile.TileContext(nc) as tc:
        with (
            contextlib.ExitStack() as _ctx,
            tc.tile_pool(name="sb", bufs=1) as sb,
            tc.tile_pool(name="ropep", bufs=2) as ropep,
            tc.tile_pool(name="ptp", bufs=8) as ptp,
            tc.tile_pool(name="recp", bufs=4) as recp,
            tc.tile_pool(name="ysbp", bufs=4) as ysbp,
        ):
            wq = sb.tile([128, DC, HPC * DH], BF16)
            wk = sb.tile([128, DC, HPC * DH], BF16)
            wv = sb.tile([128, DC, HPC * DH], BF16)
            wo = sb.tile([128, 2, D], BF16)
            cs1 = sb.tile([128, T], FP16)
            cs2 = sb.tile([128, T], FP16)
            qt = [sb.tile([128, T], BF16, tag=f"qt{p}", name=f"qt{p}") for p in range(2)]
            ktt = [sb.tile([128, T], BF16, tag=f"kt{p}", name=f"kt{p}") for p in range(2)]
            # per (ktile, head): [ones (64) | V (64 dh)] -> O' matmul yields
            # softmax denominator on partitions 0:64 (base-0 for the custom
            # fast-reciprocal op), attention numerator on 64:128
            vaug = sb.tile([128, NKT, HPC, 2 * DH], BF16)
            ot = [sb.tile([128, T], BF16, tag=f"ot{p}", name=f"ot{p}") for p in range(2)]

            # whole x resident in SBUF; all DRAM reads are contiguous.
            # spread the 7MB of input loads across 4 HWDGE queues; gpsimd
            # (slow SWDGE) only gets wo which isn't needed until ~60us.
            # queue roles: scalar = bulk x (t-chunk-major) + wk/wv;
            # sync = wq/cs then latency-critical swaps + y writes;
            # gpsimd = wo/memset/affines only.
            xt = sb.tile([128, DC, T], BF16, name="xt")
            x0_r = x_d.ap()[0].rearrange("p (c t) -> p c t", c=DC)
            nc.sync.dma_start(xt[:, 0:4, 0:TCH], x0_r[:, 0:4, :])
            nc.scalar.dma_start(xt[:, 4:8, 0:TCH], x0_r[:, 4:8, :])
            nc.sync.dma_start(wq[:], wq_d.ap().rearrange("p (c n) -> p c n", c=DC))
            nc.scalar.dma_start(wk[:], wk_d.ap().rearrange("p (c n) -> p c n", c=DC))
            nc.sync.dma_start(cs1[0:32, :], cs1_d.ap())
            nc.sync.dma_start(cs2[0:64, :], cs2_d.ap())
            nc.sync.dma_start(cs1[32:64, :], cs1[0:32, :])
            nc.sync.dma_start(cs1[64:128, :], cs1[0:64, :])
            nc.sync.dma_start(cs2[64:128, :], cs2[0:64, :])
            nc.scalar.dma_start(
                xt[:, :, TCH : 2 * TCH], x_d.ap()[1].rearrange("p (c t) -> p c t", c=DC)
            )
            nc.scalar.dma_start(wv[:], wv_d.ap().rearrange("p (c n) -> p c n", c=DC))
            for tk in range(2, 4):
                nc.scalar.dma_start(
                    xt[:, :, tk * TCH : (tk + 1) * TCH],
                    x_d.ap()[tk].rearrange("p (c t) -> p c t", c=DC),
                )
            nc.gpsimd.dma_start(wo[:], wo_d.ap().rearrange("p (c n) -> p c n", c=2))
            nc.gpsimd.memset(vaug[:, :, :, 0:DH], 1.0)

            pjps = _ctx.enter_context(tc.tile_pool(name="pjps", bufs=2, space="PSUM"))
            sps = _ctx.enter_context(tc.tile_pool(name="sps", bufs=2, space="PSUM"))
            ops = _ctx.enter_context(tc.tile_pool(name="ops", bufs=2, space="PSUM"))

            # PE warm-up: fills the input-DMA wait so HAM reaches K=8/8
            warm = sb.tile([128, TCH], BF16, name="warm")
            nc.vector.memset(warm, 0.0)
            wps = pjps.tile([128, TCH], F32, tag="pj", name="wps")
            for wi in range(32):
                nc.tensor.matmul(
                    wps[:, 0:256], warm[:, 0:128], warm[:, 0:256],
                    start=(wi == 0), stop=(wi == 31),
                )

            rope_bufs = {}

            def qk_unit(tci, w_sb, p, wi):
                # wi=0: Q projection (allocates the pair's rope tiles);
                # wi=1: K projection + strip-swap + adds -> qt[p]/ktt[p] ready
                tsl = slice(tci * TCH, (tci + 1) * TCH)
                ps = pjps.tile([128, TCH], F32, tag="pj", name=f"pj{tci}_{p}_{wi}")
                for dc in range(DC):
                    nc.tensor.matmul(
                        ps[:],
                        w_sb[:, dc, p * 128 : (p + 1) * 128],
                        xt[:, dc, tsl],
                        start=(dc == 0),
                        stop=(dc == DC - 1),
                    )
                if wi == 0:
                    t1 = ropep.tile([128, 2, TCH], FP16, tag="t1", name=f"t1_{tci}_{p}")
                    t2 = ropep.tile([128, 2, TCH], FP16, tag="t2", name=f"t2_{tci}_{p}")
                    swt = ropep.tile([128, 2, TCH], FP16, tag="swt", name=f"sw_{tci}_{p}")
                    rope_bufs[(tci, p)] = (t1, t2, swt)
                else:
                    t1, t2, swt = rope_bufs[(tci, p)]
                # fast PSUM evac (frees the pjps slot), then 16-bit DVE math
                pe = ropep.tile([128, TCH], FP16, tag="pe", name=f"pe{tci}_{p}_{wi}")
                nc.vector.tensor_copy(pe[:], ps[:])
                nc.vector.tensor_mul(t1[:, wi, :], pe[:], cs1[:, tsl])
                nc.vector.tensor_mul(t2[:, wi, :], pe[:], cs2[:, tsl])
                if wi == 1:
                    # 32-strip swap (0<->1, 2<->3) across the (q, k) pair
                    for s in range(4):
                        nc.sync.dma_start(
                            swt[s * 32 : (s + 1) * 32, :, :],
                            t2[(s ^ 1) * 32 : ((s ^ 1) + 1) * 32, :, :],
                        )
                    nc.vector.tensor_add(qt[p][:, tsl], t1[:, 0, :], swt[:, 0, :])
                    nc.vector.tensor_add(ktt[p][:, tsl], t1[:, 1, :], swt[:, 1, :])

            def v_unit(tci, tt):
                gt = tci * 4 + tt
                ps = pjps.tile([128, HPC * DH], F32, tag="pj", name=f"pjv{gt}")
                for dc in range(DC):
                    nc.tensor.matmul(
                        ps[:],
                        xt[:, dc, tci * TCH + tt * 128 : tci * TCH + (tt + 1) * 128],
                        wv[:, dc, :],
                        start=(dc == 0),
                        stop=(dc == DC - 1),
                    )
                nc.vector.tensor_copy(
                    vaug[:, gt, :, DH:],
                    ps[:].rearrange("p (h d) -> p h d", h=HPC),
                )

            def qk_pair(tci, p):
                return [
                    (lambda: qk_unit(tci, wq, p, 0)),
                    (lambda: qk_unit(tci, wk, p, 1)),
                ]

            def yproj_unit(qc, tt, ni):
                gtt = qc * 4 + tt
                alt = qc == 3 and (tt + ni) % 2 == 1
                pool, tag = (sps, "s") if alt else (pjps, "pj")
                yps = pool.tile([128, TCH], F32, tag=tag, name=f"y{gtt}_{ni}")
                for p2 in range(2):
                    nc.tensor.matmul(
                        yps[:],
                        ot[p2][:, gtt * 128 : (gtt + 1) * 128],
                        wo[:, p2, ni * TCH : (ni + 1) * TCH],
                        start=(p2 == 0),
                        stop=(p2 == 1),
                    )
                ysb = ysbp.tile([128, TCH], FP16, tag="ysb", name=f"ys{gtt}_{ni}")
                if alt:
                    nc.scalar.activation(
                        ysb[:], yps[:], mybir.ActivationFunctionType.Copy, bias=0.0
                    )
                else:
                    nc.vector.tensor_copy(ysb[:], yps[:])
                nc.sync.dma_start(
                    y_d.ap()[gtt * 128 : (gtt + 1) * 128, ni * TCH : (ni + 1) * TCH],
                    ysb[:],
                )

            def yproj_units(qc):
                return [
                    (lambda tt=tt, ni=ni: yproj_unit(qc, tt, ni))
                    for tt in range(4)
                    for ni in range(2)
                ]

            OD = 4  # O' trails S by this many k-tiles
            pending = []  # deferred O'-tail + norm from the previous hp stream

            def attn_chunk(qc, units_by_hp):
                nkt = 4 * qc + 4

                for hp in range(2):
                    units = units_by_hp[hp]
                    emitted = [0]
                    it = [0]

                    def fill():
                        it[0] += 1
                        want = len(units) * it[0] // nkt
                        while emitted[0] < want:
                            units[emitted[0]]()
                            emitted[0] += 1

                    h0, h1 = 2 * hp, 2 * hp + 1
                    opsA = ops.tile([128, TCH], F32, tag="o", name=f"oA{qc}_{hp}")
                    opsB = ops.tile([128, TCH], F32, tag="o", name=f"oB{qc}_{hp}")
                    live = {}

                    def s_step(kt):
                        off = max(0, 128 * kt - 512 * qc)
                        sp = sps.tile([128, 2, TCH], F32, tag="s", name=f"s{qc}_{hp}_{kt}")
                        pt = ptp.tile([128, 2, TCH], BF16, tag="pt", name=f"pt{qc}_{hp}_{kt}")
                        live[kt] = (pt, off)
                        for j, h in ((0, h0), (1, h1)):
                            pb = 64 * j
                            nc.tensor.matmul(
                                sp[:, j, off:],
                                ktt[hp][pb : pb + 64, kt * 128 : (kt + 1) * 128],
                                qt[hp][pb : pb + 64, qc * 512 + off : (qc + 1) * 512],
                                start=True,
                                stop=True,
                            )
                        nc.scalar.activation(
                            pt[:, :, off:], sp[:, :, off:], EXP, bias=0.0, scale=0.125
                        )
                        if kt >= 4 * qc:
                            nc.gpsimd.affine_select(
                                out=pt[:, :, off : off + 128],
                                in_=pt[:, :, off : off + 128],
                                compare_op=mybir.AluOpType.is_ge,
                                fill=0.0,
                                base=0,
                                pattern=[[0, 2], [1, 128]],
                                channel_multiplier=-1,
                            )

                    def o_step(kt, live=live, opsA=opsA, opsB=opsB,
                               h0=h0, h1=h1, nkt=nkt):
                        pt, off = live.pop(kt)
                        for j, h in ((0, h0), (1, h1)):
                            dst = opsA if j == 0 else opsB
                            nc.tensor.matmul(
                                dst[:, off:],
                                vaug[:, kt, h, :],
                                pt[:, j, off:],
                                start=(kt == 0),
                                stop=(kt == nkt - 1),
                            )

                    def norm_step(opsA=None, opsB=None, hp=hp, qc=qc):
                        qsl = slice(qc * 512, (qc + 1) * 512)
                        recA = recp.tile([64, TCH], F32, tag="rec", name=f"rA{qc}_{hp}")
                        nc.vector.reciprocal_approx_fast(recA[:], opsA[0:64, :])
                        nc.vector.tensor_mul(
                            ot[hp][0:64, qsl], opsA[64:128, :], recA[:]
                        )
                        recB = recp.tile([64, TCH], F32, tag="rec", name=f"rB{qc}_{hp}")
                        nc.vector.reciprocal_approx_fast(recB[:], opsB[0:64, :])
                        nc.vector.tensor_mul(
                            ot[hp][64:128, qsl], opsB[64:128, :], recB[:]
                        )

                    for kt in range(nkt):
                        s_step(kt)
                        for _ in range(2):
                            if pending:
                                pending.pop(0)()
                        if kt >= OD:
                            o_step(kt - OD)
                        fill()
                    while pending:
                        pending.pop(0)()
                    pending.extend(
                        [
                            (lambda kt=kt, os=o_step: os(kt))
                            for kt in range(nkt - OD, nkt)
                        ]
                        + [
                            (lambda ns=norm_step, a=opsA, b=opsB: ns(
                                opsA=a, opsB=b
                            )),
                        ]
                    )

                    while emitted[0] < len(units):
                        units[emitted[0]]()
                        emitted[0] += 1

            def V(tci, a, b):
                return [(lambda tt=tt: v_unit(tci, tt)) for tt in range(a, b)]

            yp = {qc: yproj_units(qc) for qc in range(4)}

            # minimal dense prologue: just the hp0 pair of chunk 0
            for u in qk_pair(0, 0):
                u()
            attn_chunk(0, [
                V(0, 0, 4) + qk_pair(0, 1),
                qk_pair(1, 0) + qk_pair(1, 1),
            ])
            attn_chunk(1, [
                V(1, 0, 4) + qk_pair(2, 0),
                qk_pair(2, 1) + V(2, 0, 2) + yp[0][:4],
            ])
            attn_chunk(2, [
                V(2, 2, 4) + qk_pair(3, 0) + yp[0][4:],
                qk_pair(3, 1) + V(3, 0, 2) + yp[1][:8],
            ])
            attn_chunk(3, [
                V(3, 2, 4) + yp[2][:6],
                yp[2][6:],
            ])
            while pending:
                pending.pop(0)()
            for u in yp[3]:
                u()
    nc.compile()
    return nc


_NC_CACHE = []


def _get_nc():
    if not _NC_CACHE:
        _NC_CACHE.append(_build())
    return _NC_CACHE[0]


_LAST_RESULTS = []  # stashed BassKernelResults for test harness introspection


def kernel(x, Wqkv, Wout, _trace=False, **_trace_kwargs):
    x = np.asarray(x, dtype=np.float32)
    Wqkv = np.asarray(Wqkv, dtype=np.float32)
    Wout = np.asarray(Wout, dtype=np.float32)

    nc = _get_nc()
    in_maps = []
    for c in range(NCORES):
        b, g = divmod(c, HPC)
        cols = slice(g * HPC * DH, (g + 1) * HPC * DH)
        rows = slice(g * HPC * DH, (g + 1) * HPC * DH)
        bf = ml_dtypes.bfloat16
        def _pc(a, parts=128):  # [C*parts, N] -> [parts, C*N] contiguous
            c = a.shape[0] // parts
            return np.ascontiguousarray(
                a.reshape(c, parts, -1).transpose(1, 0, 2).reshape(parts, -1)
            )

        in_maps.append(
            {
                "xp": np.ascontiguousarray(
                    x[b].T.astype(bf)
                    .reshape(DC, 128, 4, TCH)
                    .transpose(2, 1, 0, 3)
                    .reshape(4, 128, DC * TCH)
                ),
                "wq": _pc(Wqkv[:, 0:D][:, cols].astype(bf)),
                "wk": _pc(Wqkv[:, D : 2 * D][:, cols].astype(bf)),
                "wv": _pc(Wqkv[:, 2 * D : 3 * D][:, cols].astype(bf)),
                "wo": _pc(Wout[rows, :].astype(bf)),
            }
        )

    res = run_bass_kernel_spmd(
        nc, in_maps, core_ids=list(range(NCORES)), trace=_trace, **_trace_kwargs
    )
    _LAST_RESULTS.clear()
    _LAST_RESULTS.append(res)

    out = np.zeros((B, T, D), dtype=np.float32)
    for c in range(NCORES):
        b = c // HPC
        out[b] += res.results[c]["y"].astype(np.float32)
    return out
